# revision 8
# baseline (speedup 1.0000x reference)
"""Trainium2 Bass kernel for nn_BaselineBlock_SCA_Modulated.

Sharding: 8 cores = 2 batch x 4 D-slabs of 16 planes each. Halo planes are
staged host-side (zero planes at global D edges), so all cores run an
identical SPMD program. The 3x3x3 modulated depthwise conv is computed on
TensorE as a per-chunk chain of 15 matmuls fusing pw1 (C=64 -> DW=128) with
tap pairs (partition-stacked shifted copies of the LN1 output), plus one
K=9 matmul adding the pw1-bias boundary correction.
"""
import numpy as np
import ml_dtypes

C, DW, SD = 64, 128, 512
D, H, W = 64, 64, 64
NPL = 16              # output planes per core
NHALO = NPL + 2       # input planes incl halo
PW = 66               # padded row width (w in [-1, 64])
PSZ = PW * PW + 2     # padded plane size + 2 slack cols (zero)
HWC = H * W           # 4096
NCH = HWC // 128      # 32 transpose chunks / plane
EPS = 1e-6
bf = ml_dtypes.bfloat16

_CACHE = {}


def _build():
    import concourse.bacc as bacc
    import concourse.mybir as mybir
    import concourse.tile as tile
    from concourse.mybir import ActivationFunctionType as AF, AluOpType as ALU

    BF = mybir.dt.bfloat16
    F32 = mybir.dt.float32
    AX = mybir.AxisListType

    nc = bacc.Bacc("TRN2", target_bir_lowering=False, debug=False, num_devices=8)

    dram = {}
    def din(name, shape, dt=BF):
        dram[name] = nc.dram_tensor(name, shape, dt, kind="ExternalInput")
        return dram[name]

    inp_t = din("inp_t", [NHALO, C, HWC], BF)
    inp_f = din("inp_f", [NPL, C, HWC], F32)
    wpair_i = din("wpair", [128, 12, 128], BF)
    wsing_i = din("wsing", [64, 3, 128], BF)
    wcorr_i = din("wcorr", [9, NPL, 128], BF)
    ind_i = din("ind", [9, PSZ], BF)
    sd_i = din("sd", [128, 1], F32)
    modb_i = din("modb", [128, 1], F32)
    w3T_i = din("w3T", [128, 64], BF)
    scawT_i = din("scawT", [128, 128], BF)
    scab_i = din("scab", [128, 1], F32)
    beta_i = din("beta", [64, 1], F32)
    b3beta_i = din("b3beta", [64, 1], F32)
    w4T_i = din("w4T", [64, 128], BF)
    b4_i = din("b4", [128, 1], F32)
    w5T_i = din("w5T", [128, 64], BF)
    gamma_i = din("gamma", [64, 1], F32)
    b5g_i = din("b5g", [64, 1], F32)
    i128_i = din("i128", [128, 128], BF)
    i64f_i = din("i64f", [64, 64], F32)
    out_d = nc.dram_tensor("out", [NPL, C, HWC], F32, kind="ExternalOutput")

    xg_scr = nc.dram_tensor("xg_scr", [NPL, 128, HWC], BF)
    cc_a = nc.dram_tensor("cc_a", [128, 1], F32)
    cc_b = nc.dram_tensor("cc_b", [128, 1], F32)

    def geom(t, p0, pn, r0, nr, c0, ncol=64):
        """interior view [pn, nr, ncol] of a padded plane tile."""
        return t[p0:p0 + pn, 0:PW * PW].rearrange(
            "p (r w) -> p r w", w=PW)[:, r0:r0 + nr, c0:c0 + ncol]

    from contextlib import ExitStack
    with tile.TileContext(nc) as tc, ExitStack() as stk:
        cpool = stk.enter_context(tc.tile_pool(name="const", bufs=1))
        rpool = stk.enter_context(tc.tile_pool(name="ring", bufs=1))
        wpool = stk.enter_context(tc.tile_pool(name="work", bufs=2))
        xgp = stk.enter_context(tc.tile_pool(name="xg", bufs=2))
        p2p = stk.enter_context(tc.tile_pool(name="p2", bufs=1))
        psA = stk.enter_context(tc.tile_pool(name="psA", bufs=2, space="PSUM"))
        psB = stk.enter_context(tc.tile_pool(name="psB", bufs=2, space="PSUM"))

        def const(name, shape, dt):
            t = cpool.tile(shape, dt, tag=name, name=name)
            nc.sync.dma_start(t[:], dram[name][:])
            return t

        wp = const("wpair", [128, 12, 128], BF)
        ws = const("wsing", [64, 3, 128], BF)
        wc = const("wcorr", [9, NPL, 128], BF)
        ind = const("ind", [9, PSZ], BF)
        sd = const("sd", [128, 1], F32)
        modb = const("modb", [128, 1], F32)
        w3T = const("w3T", [128, 64], BF)
        scawT = const("scawT", [128, 128], BF)
        scab = const("scab", [128, 1], F32)
        beta = const("beta", [64, 1], F32)
        b3beta = const("b3beta", [64, 1], F32)
        w4T = const("w4T", [64, 128], BF)
        b4 = const("b4", [128, 1], F32)
        w5T = const("w5T", [128, 64], BF)
        gamma = const("gamma", [64, 1], F32)
        b5g = const("b5g", [64, 1], F32)
        i128 = const("i128", [128, 128], BF)
        i64f = const("i64f", [64, 64], F32)

        pools = cpool.tile([128, NPL * 8], F32, tag="pools")
        w3Tp = cpool.tile([128, 64], BF, tag="w3Tp")

        NS = 3
        t1s = [rpool.tile([128, PSZ], BF, tag=f"t1_{i}", name=f"t1_{i}") for i in range(NS)]
        ths = [rpool.tile([128, PSZ], BF, tag=f"th_{i}", name=f"th_{i}") for i in range(NS)]
        for i in range(NS):
            nc.gpsimd.memset(t1s[i][:], 0.0)
            nc.gpsimd.memset(ths[i][:], 0.0)

        # ---------------- PASS 1 ----------------
        def ln1_plane(p):
            slot = p % NS
            t1, th = t1s[slot], ths[slot]
            xT = wpool.tile([128, NCH, 64], BF, tag="xT")
            nc.sync.dma_start_transpose(xT[:], inp_t[p])
            sq = wpool.tile([128, NCH, 64], BF, tag="sq")
            nc.vector.tensor_mul(sq[:], xT[:], xT[:])
            msum = wpool.tile([128, NCH], F32, tag="msum")
            qsum = wpool.tile([128, NCH], F32, tag="qsum")
            nc.vector.tensor_reduce(msum[:], xT[:], axis=AX.X, op=ALU.add)
            nc.vector.tensor_reduce(qsum[:], sq[:], axis=AX.X, op=ALU.add)
            t1v = wpool.tile([128, NCH], F32, tag="t1v")
            nc.vector.tensor_mul(t1v[:], msum[:], msum[:])
            t3v = wpool.tile([128, NCH], F32, tag="t3v")
            nc.vector.tensor_scalar_mul(t3v[:], qsum[:], 1.0 / 63.0)
            var = wpool.tile([128, NCH], F32, tag="var")
            nc.vector.scalar_tensor_tensor(
                var[:], t1v[:], -1.0 / (64.0 * 63.0), t3v[:],
                op0=ALU.mult, op1=ALU.add)
            sv = wpool.tile([128, NCH], F32, tag="sv")
            nc.scalar.activation(sv[:], var[:], AF.Sqrt)
            nc.vector.tensor_scalar_add(sv[:], sv[:], EPS)
            rv = wpool.tile([128, NCH], F32, tag="rv")
            nc.vector.reciprocal(rv[:], sv[:])
            mrv = wpool.tile([128, NCH], F32, tag="mrv")
            nc.vector.scalar_tensor_tensor(
                mrv[:], msum[:], 1.0 / 64.0, rv[:], op0=ALU.mult, op1=ALU.mult)
            xln = wpool.tile([128, NCH, 64], BF, tag="xln")
            rvv = rv[:].unsqueeze(2).broadcast_to([128, NCH, 64])
            mrvv = mrv[:].unsqueeze(2).broadcast_to([128, NCH, 64])
            nc.vector.tensor_mul(xln[:], xT[:], rvv)
            nc.vector.tensor_sub(xln[:], xln[:], mrvv)
            for q in range(4):
                pst = psA.tile([64, 1024], BF, tag="tr")
                for g in range(8):
                    nc.tensor.transpose(
                        pst[:, g * 128:(g + 1) * 128], xln[:, 8 * q + g, :],
                        i128[:])
                nc.vector.tensor_copy(
                    geom(t1, 0, 64, 1 + 16 * q, 16, 1),
                    pst[:].rearrange("p (r w) -> p r w", w=64))
            nc.sync.dma_start(t1[64:128, 0:PSZ - 2], t1[0:64, 2:PSZ])
            nc.sync.dma_start(th[0:64, 0:PSZ - 132], t1[0:64, 132:PSZ])
            nc.sync.dma_start(th[64:128, :], t1[0:64, :])

        def conv_plane(d):
            slots = [t1s[(d + kd) % NS] for kd in range(3)]
            hslots = [ths[(d + kd) % NS] for kd in range(3)]
            xg = xgp.tile([128, HWC], BF, tag="xg")
            for cb in range(8):
                ps = psB.tile([128, 512], F32, tag="conv")
                for kd in range(3):
                    for kh in range(3):
                        nc.tensor.matmul(
                            ps[:], wp[:, kd * 3 + kh, :],
                            geom(slots[kd], 0, 128, 8 * cb + kh, 8, 0),
                            start=(kd == 0 and kh == 0), stop=False)
                for kd in range(3):
                    nc.tensor.matmul(
                        ps[:], wp[:, 9 + kd, :],
                        geom(hslots[kd], 0, 128, 8 * cb, 8, 1),
                        start=False, stop=False)
                for kd in range(3):
                    nc.tensor.matmul(
                        ps[:], ws[:, kd, :],
                        geom(slots[kd], 0, 64, 8 * cb + 1, 8, 1),
                        start=False, stop=False)
                nc.tensor.matmul(
                    ps[:], wc[:, d, :],
                    geom(ind, 0, 9, 8 * cb + 1, 8, 1),
                    start=False, stop=True)
                nc.scalar.activation(
                    xg[:, cb * 512:(cb + 1) * 512], ps[:], AF.Gelu,
                    bias=modb[:], scale=sd[:],
                    accum_out=pools[:, d * 8 + cb:d * 8 + cb + 1])
            nc.sync.dma_start(xg_scr[d], xg[:])

        for p in range(NHALO):
            ln1_plane(p)
            if p >= 2:
                conv_plane(p - 2)

        # ---------------- pooled -> gate ----------------
        pooled = cpool.tile([128, 1], F32, tag="pooled")
        nc.vector.tensor_reduce(pooled[:], pools[:], axis=AX.X, op=ALU.add)
        nc.sync.dma_start(cc_a[:], pooled[:])
        nc.gpsimd.collective_compute(
            "AllReduce", ALU.add,
            replica_groups=[[0, 1, 2, 3], [4, 5, 6, 7]],
            ins=[cc_a[:]], outs=[cc_b[:]])
        pooled2f = cpool.tile([128, 1], F32, tag="pooled2f", name="pooled2f")
        nc.sync.dma_start(pooled2f[:], cc_b[:])
        pooled2 = cpool.tile([128, 1], BF, tag="pooled2", name="pooled2")
        nc.vector.tensor_copy(pooled2[:], pooled2f[:])
        psg = psB.tile([128, 1], F32, tag="mm")
        nc.tensor.matmul(psg[:], scawT[:], pooled2[:], start=True, stop=True)
        gate = cpool.tile([128, 1], F32, tag="gatev")
        nc.scalar.activation(gate[:], psg[:], AF.Identity, bias=scab[:])
        nc.vector.tensor_scalar_mul(w3Tp[:], w3T[:], gate[:])

        # ---------------- PASS 2 ----------------
        for d in range(NPL):
            xgt = p2p.tile([128, HWC], BF, tag="xg2")
            nc.sync.dma_start(xgt[:], xg_scr[d])
            y = p2p.tile([64, HWC], F32, tag="y")
            for hf in range(2):
                ifp = p2p.tile([64, 2048], F32, tag="ifp")
                nc.sync.dma_start(ifp[:], inp_f[d][:, hf * 2048:(hf + 1) * 2048])
                for cq in range(4):
                    cb = 4 * hf + cq
                    sl = slice(cb * 512, (cb + 1) * 512)
                    ps3 = psB.tile([64, 512], F32, tag="mm")
                    nc.tensor.matmul(ps3[:], w3Tp[:], xgt[:, sl],
                                     start=True, stop=True)
                    nc.vector.affine_then_add(
                        y[:, sl], ps3[:], ifp[:, cq * 512:(cq + 1) * 512],
                        scale=beta[:], bias=b3beta[:])
            # LN2 (transposed stats)
            yTs = p2p.tile([128, NCH, 64], BF, tag="yTs")
            for hf in range(2):
                psT = psA.tile([128, 1024], F32, tag="tr")
                for g in range(16):
                    cg = 16 * hf + g
                    nc.tensor.transpose(
                        psT[:, g * 64:(g + 1) * 64],
                        y[:, cg * 128:(cg + 1) * 128], i64f[:])
                nc.vector.tensor_copy(
                    yTs[:, 16 * hf:16 * (hf + 1), :],
                    psT[:].rearrange("p (g c) -> p g c", c=64))
            sq2 = wpool.tile([128, NCH, 64], BF, tag="sq2")
            nc.vector.tensor_mul(sq2[:], yTs[:], yTs[:])
            ms2 = wpool.tile([128, NCH], F32, tag="ms2")
            qs2 = wpool.tile([128, NCH], F32, tag="qs2")
            nc.vector.tensor_reduce(ms2[:], yTs[:], axis=AX.X, op=ALU.add)
            nc.vector.tensor_reduce(qs2[:], sq2[:], axis=AX.X, op=ALU.add)
            t1v2 = wpool.tile([128, NCH], F32, tag="t1v2")
            nc.vector.tensor_mul(t1v2[:], ms2[:], ms2[:])
            t3v2 = wpool.tile([128, NCH], F32, tag="t3v2")
            nc.vector.tensor_scalar_mul(t3v2[:], qs2[:], 1.0 / 63.0)
            var2 = wpool.tile([128, NCH], F32, tag="var2")
            nc.vector.scalar_tensor_tensor(
                var2[:], t1v2[:], -1.0 / (64.0 * 63.0), t3v2[:],
                op0=ALU.mult, op1=ALU.add)
            sv2 = wpool.tile([128, NCH], F32, tag="sv2")
            nc.scalar.activation(sv2[:], var2[:], AF.Sqrt)
            nc.vector.tensor_scalar_add(sv2[:], sv2[:], EPS)
            rv2 = wpool.tile([128, NCH], F32, tag="rv2")
            nc.vector.reciprocal(rv2[:], sv2[:])
            mrv2 = wpool.tile([128, NCH], F32, tag="mrv2")
            nc.vector.scalar_tensor_tensor(
                mrv2[:], ms2[:], 1.0 / 64.0, rv2[:], op0=ALU.mult, op1=ALU.mult)
            xl2 = wpool.tile([128, NCH, 64], BF, tag="xl2")
            rvv2 = rv2[:].unsqueeze(2).broadcast_to([128, NCH, 64])
            mrvv2 = mrv2[:].unsqueeze(2).broadcast_to([128, NCH, 64])
            nc.vector.tensor_mul(xl2[:], yTs[:], rvv2)
            nc.vector.tensor_sub(xl2[:], xl2[:], mrvv2)
            xln2 = p2p.tile([64, HWC], BF, tag="xln2")
            for hf in range(2):
                psb = psA.tile([64, 2048], BF, tag="tr")
                for g in range(16):
                    nc.tensor.transpose(
                        psb[:, g * 128:(g + 1) * 128], xl2[:, 16 * hf + g, :],
                        i128[:])
                nc.vector.tensor_copy(
                    xln2[:, hf * 2048:(hf + 1) * 2048], psb[:])
            for hf in range(2):
                outp = p2p.tile([64, 2048], F32, tag="outp")
                xg2 = p2p.tile([128, 2048], BF, tag="xg2b")
                for cq in range(4):
                    cb = 4 * hf + cq
                    sl = slice(cb * 512, (cb + 1) * 512)
                    lsl = slice(cq * 512, (cq + 1) * 512)
                    ps4 = psB.tile([128, 512], F32, tag="mm")
                    nc.tensor.matmul(ps4[:], w4T[:], xln2[:, sl],
                                     start=True, stop=True)
                    nc.scalar.activation(xg2[:, lsl], ps4[:], AF.Gelu, bias=b4[:])
                    ps5 = psB.tile([64, 512], F32, tag="mm")
                    nc.tensor.matmul(ps5[:], w5T[:], xg2[:, lsl],
                                     start=True, stop=True)
                    nc.vector.affine_then_add(
                        outp[:, lsl], ps5[:], y[:, sl],
                        scale=gamma[:], bias=b5g[:])
                nc.sync.dma_start(
                    out_d[d][:, hf * 2048:(hf + 1) * 2048], outp[:])

    nc.compile()
    return nc


def _host_prep(inputs):
    """Per-core input maps + reassembly metadata. All folds in fp32 numpy."""
    inp = np.asarray(inputs["inp"], np.float32)
    style = np.asarray(inputs["style_vector"], np.float32)
    w1 = np.asarray(inputs["w1"], np.float32)
    b1 = np.asarray(inputs["b1"], np.float32)
    mod_w = np.asarray(inputs["mod_w"], np.float32)
    mod_b = np.asarray(inputs["mod_b"], np.float32)
    style_w = np.asarray(inputs["style_w"], np.float32)
    style_b = np.asarray(inputs["style_b"], np.float32)
    sca_w = np.asarray(inputs["sca_w"], np.float32)
    sca_b = np.asarray(inputs["sca_b"], np.float32)
    w3 = np.asarray(inputs["w3"], np.float32)
    b3 = np.asarray(inputs["b3"], np.float32)
    w4 = np.asarray(inputs["w4"], np.float32)
    b4 = np.asarray(inputs["b4"], np.float32)
    w5 = np.asarray(inputs["w5"], np.float32)
    b5 = np.asarray(inputs["b5"], np.float32)
    ln1_w = np.asarray(inputs["ln1_w"], np.float32).reshape(C)
    ln2_w = np.asarray(inputs["ln2_w"], np.float32).reshape(C)
    beta = np.asarray(inputs["beta"], np.float32).reshape(C)
    gamma = np.asarray(inputs["gamma"], np.float32).reshape(C)

    # style modulation (exact, host fp32)
    s = style @ style_w.T + style_b                     # [B, DW]
    k2 = (mod_w ** 2).sum(axis=(1, 2, 3, 4))            # [DW]
    demod = 1.0 / np.sqrt(k2[None] * s * s + 1e-8)      # [B, DW]
    sdv = s * demod                                     # [B, DW]

    W1t = w1 * ln1_w[None, :]                           # [DW, C]
    wdw = mod_w[:, 0]                                   # [DW, 3,3,3]

    # 12 pair matmuls: lhsT [128, 128]: rows 0-63 tap A via x_ln, 64-127 tap B
    wpair = np.zeros((128, 12, 128), np.float32)
    for kd in range(3):
        for kh in range(3):
            i = kd * 3 + kh
            wpair[0:64, i, :] = (W1t * wdw[:, kd, kh, 0][:, None]).T
            wpair[64:128, i, :] = (W1t * wdw[:, kd, kh, 2][:, None]).T
    for kd in range(3):
        # TH tile: lower = x_ln shifted +2 rows (tap kh=+1), upper = plain
        # (tap kh=-1), both at kw=0 (index 1)
        wpair[0:64, 9 + kd, :] = (W1t * wdw[:, kd, 2, 1][:, None]).T
        wpair[64:128, 9 + kd, :] = (W1t * wdw[:, kd, 0, 1][:, None]).T
    wsing = np.zeros((64, 3, 128), np.float32)
    for kd in range(3):
        wsing[:, kd, :] = (W1t * wdw[:, kd, 1, 1][:, None]).T

    # boundary-correction coefficients (b1 * sum of valid taps)
    def S(cd, ch, cw):
        vd = {0: [1, 2], 1: [0, 1, 2], 2: [0, 1]}[cd]
        vh = {0: [1, 2], 1: [0, 1, 2], 2: [0, 1]}[ch]
        vw = {0: [1, 2], 1: [0, 1, 2], 2: [0, 1]}[cw]
        return wdw[:, vd][:, :, vh][:, :, :, vw].sum(axis=(1, 2, 3))  # [DW]

    ind = np.zeros((9, PSZ), np.float32)
    g = np.zeros((9, PW, PW), np.float32)
    g[0, 1:65, 1:65] = 1.0
    g[1, 1, 1:65] = 1.0       # h = 0
    g[2, 64, 1:65] = 1.0      # h = 63
    g[3, 1:65, 1] = 1.0       # w = 0
    g[4, 1:65, 64] = 1.0      # w = 63
    g[5, 1, 1] = 1.0
    g[6, 1, 64] = 1.0
    g[7, 64, 1] = 1.0
    g[8, 64, 64] = 1.0
    ind[:, 0:PW * PW] = g.reshape(9, -1)

    def corr_for(dcase):
        c = np.zeros((9, 128), np.float32)
        base = S(dcase, 1, 1)
        ch0 = S(dcase, 0, 1) - base
        ch1 = S(dcase, 2, 1) - base
        cw0 = S(dcase, 1, 0) - base
        cw1 = S(dcase, 1, 2) - base
        c[0] = base
        c[1], c[2], c[3], c[4] = ch0, ch1, cw0, cw1
        c[5] = S(dcase, 0, 0) - S(dcase, 0, 1) - S(dcase, 1, 0) + base
        c[6] = S(dcase, 0, 2) - S(dcase, 0, 1) - S(dcase, 1, 2) + base
        c[7] = S(dcase, 2, 0) - S(dcase, 2, 1) - S(dcase, 1, 0) + base
        c[8] = S(dcase, 2, 2) - S(dcase, 2, 1) - S(dcase, 1, 2) + base
        return c * b1[None, :]

    corr_tab = {c: corr_for(c) for c in (0, 1, 2)}

    common = dict(
        wpair=wpair.astype(bf), wsing=wsing.astype(bf),
        ind=ind.astype(bf),
        modb=mod_b.reshape(128, 1).astype(np.float32),
        w3T=w3.T.astype(bf),
        scawT=(sca_w.T / float(D * H * W)).astype(bf),
        scab=sca_b.reshape(128, 1).astype(np.float32),
        beta=beta.reshape(64, 1), b3beta=(b3 * beta).reshape(64, 1),
        w4T=(w4 * ln2_w[None, :]).T.astype(bf),
        b4=b4.reshape(128, 1).astype(np.float32),
        w5T=w5.T.astype(bf),
        gamma=gamma.reshape(64, 1), b5g=(b5 * gamma).reshape(64, 1),
        i128=np.eye(128, dtype=np.float32).astype(bf),
        i64f=np.eye(64, dtype=np.float32),
    )

    in_maps = []
    for k in range(8):
        b, d0 = k // 4, (k % 4) * NPL
        ip = inp[b]                                     # [C, D, H, W]
        halo = np.zeros((NHALO, C, HWC), np.float32)
        lo, hi = max(d0 - 1, 0), min(d0 + NPL + 1, D)
        halo[lo - (d0 - 1):hi - (d0 - 1)] = (
            ip[:, lo:hi].transpose(1, 0, 2, 3).reshape(hi - lo, C, HWC))
        wcorr = np.zeros((9, NPL, 128), np.float32)
        for i in range(NPL):
            dg = d0 + i
            dcase = 0 if dg == 0 else (2 if dg == D - 1 else 1)
            wcorr[:, i, :] = corr_tab[dcase]
        m = dict(common)
        m["inp_t"] = halo.astype(bf)
        m["inp_f"] = np.ascontiguousarray(
            ip[:, d0:d0 + NPL].transpose(1, 0, 2, 3).reshape(NPL, C, HWC))
        m["wcorr"] = wcorr.astype(bf)
        m["sd"] = sdv[b].reshape(128, 1).astype(np.float32)
        in_maps.append(m)
    return in_maps


def kernel(**inputs):
    from concourse.bass_utils import run_bass_kernel_spmd
    if "nc" not in _CACHE:
        _CACHE["nc"] = _build()
    nc = _CACHE["nc"]
    in_maps = _host_prep(inputs)
    res = run_bass_kernel_spmd(nc, in_maps, list(range(8)))
    _CACHE["last_res"] = res
    out = np.empty((2, C, D, H, W), np.float32)
    for k in range(8):
        b, d0 = k // 4, (k % 4) * NPL
        o = res.results[k]["out"]                       # [NPL, C, HWC]
        out[b, :, d0:d0 + NPL] = o.reshape(NPL, C, H, W).transpose(1, 0, 2, 3)
    return out


# revision 14
# speedup vs baseline: 7285.1535x; 7285.1535x over previous
"""Trainium2 Bass kernel for nn_BaselineBlock_SCA_Modulated.

Sharding: 8 cores = 2 batch x 4 D-slabs of 16 planes each. Halo planes are
staged host-side (zero planes at global D edges), so all cores run an
identical SPMD program. The 3x3x3 modulated depthwise conv is computed on
TensorE as a per-chunk chain of 15 matmuls fusing pw1 (C=64 -> DW=128) with
tap pairs (partition-stacked shifted copies of the LN1 output), plus one
K=9 matmul adding the pw1-bias boundary correction.
"""
import numpy as np
import ml_dtypes

C, DW, SD = 64, 128, 512
D, H, W = 64, 64, 64
NPL = 16              # output planes per core
NHALO = NPL + 2       # input planes incl halo
PW = 66               # padded row width (w in [-1, 64])
PSZ = PW * PW + 2     # padded plane size + 2 slack cols (zero)
HWC = H * W           # 4096
NCH = HWC // 128      # 32 transpose chunks / plane
EPS = 1e-6
bf = ml_dtypes.bfloat16

_CACHE = {}


def _build():
    import concourse.bacc as bacc
    import concourse.mybir as mybir
    import concourse.tile as tile
    from concourse.mybir import ActivationFunctionType as AF, AluOpType as ALU

    BF = mybir.dt.bfloat16
    F32 = mybir.dt.float32
    AX = mybir.AxisListType

    nc = bacc.Bacc("TRN2", target_bir_lowering=False, debug=False, num_devices=8)

    dram = {}
    def din(name, shape, dt=BF):
        dram[name] = nc.dram_tensor(name, shape, dt, kind="ExternalInput")
        return dram[name]

    inp_t = din("inp_t", [NHALO, C, HWC], BF)
    inp_f = din("inp_f", [NPL, C, HWC], F32)
    wpair_i = din("wpair", [128, 12, 128], BF)
    wsing_i = din("wsing", [64, 3, 128], BF)
    wcorr_i = din("wcorr", [9, NPL, 128], BF)
    ind_i = din("ind", [9, 3, 512], BF)
    sd_i = din("sd", [128, 1], F32)
    modb_i = din("modb", [128, 1], F32)
    w3T_i = din("w3T", [128, 64], BF)
    scawT_i = din("scawT", [128, 128], BF)
    scab_i = din("scab", [128, 1], F32)
    beta_i = din("beta", [64, 1], F32)
    b3beta_i = din("b3beta", [64, 1], F32)
    w4T_i = din("w4T", [64, 128], BF)
    b4_i = din("b4", [128, 1], F32)
    w5T_i = din("w5T", [128, 64], BF)
    gamma_i = din("gamma", [64, 1], F32)
    b5g_i = din("b5g", [64, 1], F32)
    i128_i = din("i128", [128, 128], BF)
    i64f_i = din("i64f", [64, 64], F32)
    out_d = nc.dram_tensor("out", [NPL, C, HWC], F32, kind="ExternalOutput")

    xg_scr = nc.dram_tensor("xg_scr", [NPL, 128, HWC], BF)
    cc_a = nc.dram_tensor("cc_a", [128, 1], F32)
    cc_b = nc.dram_tensor("cc_b", [128, 1], F32)

    def geom(t, p0, pn, r0, nr, c0, ncol=64):
        """interior view [pn, nr, ncol] of a padded plane tile."""
        return t[p0:p0 + pn, 0:PW * PW].rearrange(
            "p (r w) -> p r w", w=PW)[:, r0:r0 + nr, c0:c0 + ncol]

    from contextlib import ExitStack
    with tile.TileContext(nc) as tc, ExitStack() as stk:
        cpool = stk.enter_context(tc.tile_pool(name="const", bufs=1))
        rpool = stk.enter_context(tc.tile_pool(name="ring", bufs=1))
        wpool = stk.enter_context(tc.tile_pool(name="work", bufs=2))
        xgp = stk.enter_context(tc.tile_pool(name="xg", bufs=2))
        p2p = stk.enter_context(tc.tile_pool(name="p2", bufs=1))
        psA = stk.enter_context(tc.tile_pool(name="psA", bufs=2, space="PSUM"))
        psB = stk.enter_context(tc.tile_pool(name="psB", bufs=2, space="PSUM"))

        def const(name, shape, dt):
            t = cpool.tile(shape, dt, tag=name, name=name)
            nc.sync.dma_start(t[:], dram[name][:])
            return t

        wp = const("wpair", [128, 12, 128], BF)
        ws = const("wsing", [64, 3, 128], BF)
        wc = const("wcorr", [9, NPL, 128], BF)
        ind = const("ind", [9, 3, 512], BF)
        sd = const("sd", [128, 1], F32)
        modb = const("modb", [128, 1], F32)
        w3T = const("w3T", [128, 64], BF)
        scawT = const("scawT", [128, 128], BF)
        scab = const("scab", [128, 1], F32)
        beta = const("beta", [64, 1], F32)
        b3beta = const("b3beta", [64, 1], F32)
        w4T = const("w4T", [64, 128], BF)
        b4 = const("b4", [128, 1], F32)
        w5T = const("w5T", [128, 64], BF)
        gamma = const("gamma", [64, 1], F32)
        b5g = const("b5g", [64, 1], F32)
        i128 = const("i128", [128, 128], BF)
        i64f = const("i64f", [64, 64], F32)

        pools = cpool.tile([128, NPL * 8], F32, tag="pools")
        w3Tp = cpool.tile([128, 64], BF, tag="w3Tp")

        NS = 4
        t1s = [rpool.tile([128, PSZ], BF, tag=f"t1_{i}", name=f"t1_{i}") for i in range(NS)]
        ths = [rpool.tile([128, PSZ], BF, tag=f"th_{i}", name=f"th_{i}") for i in range(NS)]
        for i in range(NS):
            nc.gpsimd.memset(t1s[i][:], 0.0)
            nc.gpsimd.memset(ths[i][:], 0.0)

        # ---------------- PASS 1 ----------------
        def ln1_plane(p):
            slot = p % NS
            t1, th = t1s[slot], ths[slot]
            xT = wpool.tile([128, NCH, 64], BF, tag="xT")
            nc.sync.dma_start_transpose(xT[:], inp_t[p])
            sq = wpool.tile([128, NCH, 64], BF, tag="sq", bufs=1)
            nc.vector.tensor_mul(sq[:], xT[:], xT[:])
            msum = wpool.tile([128, NCH], F32, tag="msum")
            qsum = wpool.tile([128, NCH], F32, tag="qsum")
            nc.vector.tensor_reduce(msum[:], xT[:], axis=AX.X, op=ALU.add)
            nc.vector.tensor_reduce(qsum[:], sq[:], axis=AX.X, op=ALU.add)
            t1v = wpool.tile([128, NCH], F32, tag="t1v")
            nc.vector.tensor_mul(t1v[:], msum[:], msum[:])
            t3v = wpool.tile([128, NCH], F32, tag="t3v")
            nc.vector.tensor_scalar_mul(t3v[:], qsum[:], 1.0 / 63.0)
            var = wpool.tile([128, NCH], F32, tag="var")
            nc.vector.scalar_tensor_tensor(
                var[:], t1v[:], -1.0 / (64.0 * 63.0), t3v[:],
                op0=ALU.mult, op1=ALU.add)
            sv = wpool.tile([128, NCH], F32, tag="sv")
            nc.scalar.activation(sv[:], var[:], AF.Sqrt)
            nc.vector.tensor_scalar_add(sv[:], sv[:], EPS)
            rv = wpool.tile([128, NCH], F32, tag="rv")
            nc.vector.reciprocal(rv[:], sv[:])
            mrv = wpool.tile([128, NCH], F32, tag="mrv")
            nc.vector.scalar_tensor_tensor(
                mrv[:], msum[:], 1.0 / 64.0, rv[:], op0=ALU.mult, op1=ALU.mult)
            xln = xT
            rvv = rv[:].unsqueeze(2).broadcast_to([128, NCH, 64])
            mrvv = mrv[:].unsqueeze(2).broadcast_to([128, NCH, 64])
            nc.vector.tensor_mul(xln[:], xT[:], rvv)
            nc.vector.tensor_sub(xln[:], xln[:], mrvv)
            for q in range(2):
                pst = psA.tile([64, 2048], BF, tag="tr")
                for g in range(16):
                    nc.tensor.transpose(
                        pst[:, g * 128:(g + 1) * 128], xln[:, 16 * q + g, :],
                        i128[:])
                nc.vector.tensor_copy(
                    geom(t1, 0, 64, 1 + 32 * q, 32, 1),
                    pst[:].rearrange("p (r w) -> p r w", w=64))
            nc.sync.dma_start(t1[64:128, 0:PSZ - 2], t1[0:64, 2:PSZ])
            nc.sync.dma_start(th[0:64, 0:PSZ - 132], t1[0:64, 132:PSZ])
            nc.sync.dma_start(th[64:128, :], t1[0:64, :])

        def conv_plane(d):
            slots = [t1s[(d + kd) % NS] for kd in range(3)]
            hslots = [ths[(d + kd) % NS] for kd in range(3)]
            xg = xgp.tile([128, HWC], BF, tag="xg")
            for cb in range(8):
                ps = psB.tile([128, 512], F32, tag="conv")
                for kd in range(3):
                    for kh in range(3):
                        nc.tensor.matmul(
                            ps[:], wp[:, kd * 3 + kh, :],
                            geom(slots[kd], 0, 128, 8 * cb + kh, 8, 0),
                            start=(kd == 0 and kh == 0), stop=False)
                for kd in range(3):
                    nc.tensor.matmul(
                        ps[:], wp[:, 9 + kd, :],
                        geom(hslots[kd], 0, 128, 8 * cb, 8, 1),
                        start=False, stop=False)
                for kd in range(3):
                    nc.tensor.matmul(
                        ps[:], ws[:, kd, :],
                        geom(slots[kd], 0, 64, 8 * cb + 1, 8, 1),
                        start=False, stop=False)
                pat = 0 if cb == 0 else (2 if cb == 7 else 1)
                nc.tensor.matmul(
                    ps[:], wc[:, d, :], ind[:, pat, :],
                    start=False, stop=True)
                nc.scalar.activation(
                    xg[:, cb * 512:(cb + 1) * 512], ps[:], AF.Gelu,
                    bias=modb[:], scale=sd[:],
                    accum_out=pools[:, d * 8 + cb:d * 8 + cb + 1])
            nc.sync.dma_start(xg_scr[d], xg[:])

        for p in range(NHALO):
            ln1_plane(p)
            if p >= 2:
                conv_plane(p - 2)

        # ---------------- pooled -> gate ----------------
        pooled = cpool.tile([128, 1], F32, tag="pooled")
        nc.vector.tensor_reduce(pooled[:], pools[:], axis=AX.X, op=ALU.add)
        nc.sync.dma_start(cc_a[:], pooled[:])
        nc.gpsimd.collective_compute(
            "AllReduce", ALU.add,
            replica_groups=[[0, 1, 2, 3], [4, 5, 6, 7]],
            ins=[cc_a[:]], outs=[cc_b[:]])
        pooled2f = cpool.tile([128, 1], F32, tag="pooled2f", name="pooled2f")
        nc.sync.dma_start(pooled2f[:], cc_b[:])
        pooled2 = cpool.tile([128, 1], BF, tag="pooled2", name="pooled2")
        nc.vector.tensor_copy(pooled2[:], pooled2f[:])
        psg = psB.tile([128, 1], F32, tag="mm")
        nc.tensor.matmul(psg[:], scawT[:], pooled2[:], start=True, stop=True)
        gate = cpool.tile([128, 1], F32, tag="gatev")
        nc.scalar.activation(gate[:], psg[:], AF.Identity, bias=scab[:])
        nc.vector.tensor_scalar_mul(w3Tp[:], w3T[:], gate[:])

        # ---------------- PASS 2 ----------------
        for d in range(NPL):
            xgt = p2p.tile([128, HWC], BF, tag="xg2")
            nc.sync.dma_start(xgt[:], xg_scr[d])
            y = p2p.tile([64, HWC], F32, tag="y", bufs=2)
            for hf in range(2):
                ifp = p2p.tile([64, 2048], F32, tag="ifp")
                nc.sync.dma_start(ifp[:], inp_f[d][:, hf * 2048:(hf + 1) * 2048])
                for cq in range(4):
                    cb = 4 * hf + cq
                    sl = slice(cb * 512, (cb + 1) * 512)
                    ps3 = psB.tile([64, 512], F32, tag="mm")
                    nc.tensor.matmul(ps3[:], w3Tp[:], xgt[:, sl],
                                     start=True, stop=True)
                    nc.vector.affine_then_add(
                        y[:, sl], ps3[:], ifp[:, cq * 512:(cq + 1) * 512],
                        scale=beta[:], bias=b3beta[:])
            # LN2 (transposed stats)
            yTs = p2p.tile([128, NCH, 64], BF, tag="yTs")
            for hf in range(2):
                psT = psA.tile([128, 1024], F32, tag="tr")
                for g in range(16):
                    cg = 16 * hf + g
                    nc.tensor.transpose(
                        psT[:, g * 64:(g + 1) * 64],
                        y[:, cg * 128:(cg + 1) * 128], i64f[:])
                nc.vector.tensor_copy(
                    yTs[:, 16 * hf:16 * (hf + 1), :],
                    psT[:].rearrange("p (g c) -> p g c", c=64))
            sq2 = wpool.tile([128, NCH, 64], BF, tag="sq2", bufs=1)
            nc.vector.tensor_mul(sq2[:], yTs[:], yTs[:])
            ms2 = wpool.tile([128, NCH], F32, tag="ms2")
            qs2 = wpool.tile([128, NCH], F32, tag="qs2")
            nc.vector.tensor_reduce(ms2[:], yTs[:], axis=AX.X, op=ALU.add)
            nc.vector.tensor_reduce(qs2[:], sq2[:], axis=AX.X, op=ALU.add)
            t1v2 = wpool.tile([128, NCH], F32, tag="t1v2")
            nc.vector.tensor_mul(t1v2[:], ms2[:], ms2[:])
            t3v2 = wpool.tile([128, NCH], F32, tag="t3v2")
            nc.vector.tensor_scalar_mul(t3v2[:], qs2[:], 1.0 / 63.0)
            var2 = wpool.tile([128, NCH], F32, tag="var2")
            nc.vector.scalar_tensor_tensor(
                var2[:], t1v2[:], -1.0 / (64.0 * 63.0), t3v2[:],
                op0=ALU.mult, op1=ALU.add)
            sv2 = wpool.tile([128, NCH], F32, tag="sv2")
            nc.scalar.activation(sv2[:], var2[:], AF.Sqrt)
            nc.vector.tensor_scalar_add(sv2[:], sv2[:], EPS)
            rv2 = wpool.tile([128, NCH], F32, tag="rv2")
            nc.vector.reciprocal(rv2[:], sv2[:])
            mrv2 = wpool.tile([128, NCH], F32, tag="mrv2")
            nc.vector.scalar_tensor_tensor(
                mrv2[:], ms2[:], 1.0 / 64.0, rv2[:], op0=ALU.mult, op1=ALU.mult)
            xl2 = yTs
            rvv2 = rv2[:].unsqueeze(2).broadcast_to([128, NCH, 64])
            mrvv2 = mrv2[:].unsqueeze(2).broadcast_to([128, NCH, 64])
            nc.vector.tensor_mul(xl2[:], yTs[:], rvv2)
            nc.vector.tensor_sub(xl2[:], xl2[:], mrvv2)
            xln2 = p2p.tile([64, HWC], BF, tag="xln2")
            for hf in range(2):
                psb = psA.tile([64, 2048], BF, tag="tr")
                for g in range(16):
                    nc.tensor.transpose(
                        psb[:, g * 128:(g + 1) * 128], xl2[:, 16 * hf + g, :],
                        i128[:])
                nc.vector.tensor_copy(
                    xln2[:, hf * 2048:(hf + 1) * 2048], psb[:])
            for hf in range(2):
                outp = p2p.tile([64, 2048], F32, tag="outp")
                xg2 = p2p.tile([128, 2048], BF, tag="xg2b")
                for cq in range(4):
                    cb = 4 * hf + cq
                    sl = slice(cb * 512, (cb + 1) * 512)
                    lsl = slice(cq * 512, (cq + 1) * 512)
                    ps4 = psB.tile([128, 512], F32, tag="mm")
                    nc.tensor.matmul(ps4[:], w4T[:], xln2[:, sl],
                                     start=True, stop=True)
                    nc.scalar.activation(xg2[:, lsl], ps4[:], AF.Gelu, bias=b4[:])
                    ps5 = psB.tile([64, 512], F32, tag="mm")
                    nc.tensor.matmul(ps5[:], w5T[:], xg2[:, lsl],
                                     start=True, stop=True)
                    nc.vector.affine_then_add(
                        outp[:, lsl], ps5[:], y[:, sl],
                        scale=gamma[:], bias=b5g[:])
                nc.sync.dma_start(
                    out_d[d][:, hf * 2048:(hf + 1) * 2048], outp[:])

    nc.compile()
    return nc


def _host_prep(inputs):
    """Per-core input maps + reassembly metadata. All folds in fp32 numpy."""
    inp = np.asarray(inputs["inp"], np.float32)
    style = np.asarray(inputs["style_vector"], np.float32)
    w1 = np.asarray(inputs["w1"], np.float32)
    b1 = np.asarray(inputs["b1"], np.float32)
    mod_w = np.asarray(inputs["mod_w"], np.float32)
    mod_b = np.asarray(inputs["mod_b"], np.float32)
    style_w = np.asarray(inputs["style_w"], np.float32)
    style_b = np.asarray(inputs["style_b"], np.float32)
    sca_w = np.asarray(inputs["sca_w"], np.float32)
    sca_b = np.asarray(inputs["sca_b"], np.float32)
    w3 = np.asarray(inputs["w3"], np.float32)
    b3 = np.asarray(inputs["b3"], np.float32)
    w4 = np.asarray(inputs["w4"], np.float32)
    b4 = np.asarray(inputs["b4"], np.float32)
    w5 = np.asarray(inputs["w5"], np.float32)
    b5 = np.asarray(inputs["b5"], np.float32)
    ln1_w = np.asarray(inputs["ln1_w"], np.float32).reshape(C)
    ln2_w = np.asarray(inputs["ln2_w"], np.float32).reshape(C)
    beta = np.asarray(inputs["beta"], np.float32).reshape(C)
    gamma = np.asarray(inputs["gamma"], np.float32).reshape(C)

    # style modulation (exact, host fp32)
    s = style @ style_w.T + style_b                     # [B, DW]
    k2 = (mod_w ** 2).sum(axis=(1, 2, 3, 4))            # [DW]
    demod = 1.0 / np.sqrt(k2[None] * s * s + 1e-8)      # [B, DW]
    sdv = s * demod                                     # [B, DW]

    W1t = w1 * ln1_w[None, :]                           # [DW, C]
    wdw = mod_w[:, 0]                                   # [DW, 3,3,3]

    # 12 pair matmuls: lhsT [128, 128]: rows 0-63 tap A via x_ln, 64-127 tap B
    wpair = np.zeros((128, 12, 128), np.float32)
    for kd in range(3):
        for kh in range(3):
            i = kd * 3 + kh
            wpair[0:64, i, :] = (W1t * wdw[:, kd, kh, 0][:, None]).T
            wpair[64:128, i, :] = (W1t * wdw[:, kd, kh, 2][:, None]).T
    for kd in range(3):
        # TH tile: lower = x_ln shifted +2 rows (tap kh=+1), upper = plain
        # (tap kh=-1), both at kw=0 (index 1)
        wpair[0:64, 9 + kd, :] = (W1t * wdw[:, kd, 2, 1][:, None]).T
        wpair[64:128, 9 + kd, :] = (W1t * wdw[:, kd, 0, 1][:, None]).T
    wsing = np.zeros((64, 3, 128), np.float32)
    for kd in range(3):
        wsing[:, kd, :] = (W1t * wdw[:, kd, 1, 1][:, None]).T

    # boundary-correction coefficients (b1 * sum of valid taps)
    def S(cd, ch, cw):
        vd = {0: [1, 2], 1: [0, 1, 2], 2: [0, 1]}[cd]
        vh = {0: [1, 2], 1: [0, 1, 2], 2: [0, 1]}[ch]
        vw = {0: [1, 2], 1: [0, 1, 2], 2: [0, 1]}[cw]
        return wdw[:, vd][:, :, vh][:, :, :, vw].sum(axis=(1, 2, 3))  # [DW]

    g = np.zeros((9, 64, 64), np.float32)
    g[0] = 1.0
    g[1, 0, :] = 1.0          # h = 0
    g[2, 63, :] = 1.0         # h = 63
    g[3, :, 0] = 1.0          # w = 0
    g[4, :, 63] = 1.0         # w = 63
    g[5, 0, 0] = 1.0
    g[6, 0, 63] = 1.0
    g[7, 63, 0] = 1.0
    g[8, 63, 63] = 1.0
    ind = np.zeros((9, 3, 512), np.float32)
    ind[:, 0] = g[:, 0:8, :].reshape(9, -1)     # chunk 0 (h=0 edge)
    ind[:, 1] = g[:, 8:16, :].reshape(9, -1)    # generic middle chunk
    ind[:, 2] = g[:, 56:64, :].reshape(9, -1)   # chunk 7 (h=63 edge)

    def corr_for(dcase):
        c = np.zeros((9, 128), np.float32)
        base = S(dcase, 1, 1)
        ch0 = S(dcase, 0, 1) - base
        ch1 = S(dcase, 2, 1) - base
        cw0 = S(dcase, 1, 0) - base
        cw1 = S(dcase, 1, 2) - base
        c[0] = base
        c[1], c[2], c[3], c[4] = ch0, ch1, cw0, cw1
        c[5] = S(dcase, 0, 0) - S(dcase, 0, 1) - S(dcase, 1, 0) + base
        c[6] = S(dcase, 0, 2) - S(dcase, 0, 1) - S(dcase, 1, 2) + base
        c[7] = S(dcase, 2, 0) - S(dcase, 2, 1) - S(dcase, 1, 0) + base
        c[8] = S(dcase, 2, 2) - S(dcase, 2, 1) - S(dcase, 1, 2) + base
        return c * b1[None, :]

    corr_tab = {c: corr_for(c) for c in (0, 1, 2)}

    common = dict(
        wpair=wpair.astype(bf), wsing=wsing.astype(bf),
        ind=ind.astype(bf),
        modb=mod_b.reshape(128, 1).astype(np.float32),
        w3T=w3.T.astype(bf),
        scawT=(sca_w.T / float(D * H * W)).astype(bf),
        scab=sca_b.reshape(128, 1).astype(np.float32),
        beta=beta.reshape(64, 1), b3beta=(b3 * beta).reshape(64, 1),
        w4T=(w4 * ln2_w[None, :]).T.astype(bf),
        b4=b4.reshape(128, 1).astype(np.float32),
        w5T=w5.T.astype(bf),
        gamma=gamma.reshape(64, 1), b5g=(b5 * gamma).reshape(64, 1),
        i128=np.eye(128, dtype=np.float32).astype(bf),
        i64f=np.eye(64, dtype=np.float32),
    )

    in_maps = []
    for k in range(8):
        b, d0 = k // 4, (k % 4) * NPL
        ip = inp[b]                                     # [C, D, H, W]
        halo = np.zeros((NHALO, C, HWC), np.float32)
        lo, hi = max(d0 - 1, 0), min(d0 + NPL + 1, D)
        halo[lo - (d0 - 1):hi - (d0 - 1)] = (
            ip[:, lo:hi].transpose(1, 0, 2, 3).reshape(hi - lo, C, HWC))
        wcorr = np.zeros((9, NPL, 128), np.float32)
        for i in range(NPL):
            dg = d0 + i
            dcase = 0 if dg == 0 else (2 if dg == D - 1 else 1)
            wcorr[:, i, :] = corr_tab[dcase]
        m = dict(common)
        m["inp_t"] = halo.astype(bf)
        m["inp_f"] = np.ascontiguousarray(
            ip[:, d0:d0 + NPL].transpose(1, 0, 2, 3).reshape(NPL, C, HWC))
        m["wcorr"] = wcorr.astype(bf)
        m["sd"] = sdv[b].reshape(128, 1).astype(np.float32)
        in_maps.append(m)
    return in_maps


def kernel(**inputs):
    from concourse.bass_utils import run_bass_kernel_spmd
    if "nc" not in _CACHE:
        _CACHE["nc"] = _build()
    nc = _CACHE["nc"]
    in_maps = _host_prep(inputs)
    res = run_bass_kernel_spmd(nc, in_maps, list(range(8)))
    _CACHE["last_res"] = res
    out = np.empty((2, C, D, H, W), np.float32)
    for k in range(8):
        b, d0 = k // 4, (k % 4) * NPL
        o = res.results[k]["out"]                       # [NPL, C, HWC]
        out[b, :, d0:d0 + NPL] = o.reshape(NPL, C, H, W).transpose(1, 0, 2, 3)
    return out
